# revision 23
# baseline (speedup 1.0000x reference)
"""MultiHeadGeneralizedPooling Trainium2 kernel.

Data-parallel over batch: 32 batches -> 8 cores x 4 batches.
Per core, everything is computed in "feature-major" layout (feature dim on
SBUF partitions, sequence on the free axis):

  Hi^T (d, s)  = P_cat^T @ X^T        TensorE bf16, PSUM (no bias: P_b folded)
  A1^T (dh, s) = relu(W1aug^T @ Hi^T) K=97 (97th row = ones -> W1_b + P_b@W1)
  A2^T (d, s)  = W2^T @ A1^T          accumulated over 3 k-tiles
  E            = exp(A2^T + W2_b)     ScalarE; accum_out -> Z partial
  u[d]         = sum_s E * Hi         sc-pair scalar_tensor_tensor accum
  v            = u / Z + P_b          tiny (96, 32) chain at the end

Schedule: sc-major projection sweeps; remap (flat -> per-head layout) via
SBUF->SBUF DMA at (dt, sc) granularity so MLP quarters unlock per-sc; MLP
units are stage-split (W1+relu | W2+exp+stt) and software-pipelined; xt
prefetch rides the gpsimd DMA queue so remaps never queue behind it.
Output is [96, 32] (host transposes).
"""

import numpy as np
from contextlib import ExitStack

B, S, T = 32, 2048, 768
NH, DH, DHID = 8, 96, 384
NCORES = 8
BPC = B // NCORES  # batches per core
KT = T // 128      # 6 contraction tiles
DT = (NH * DH) // 128  # 6 d-tiles of the packed head dim
SC = 4             # s-chunks per batch
SCW = S // SC      # 512
KC = DHID // 128   # 3
import os
N_WARM = int(os.environ.get("K_NWARM", "42"))  # PE clock-gate warmup matmuls
RELU_SPLIT = os.environ.get("K_RELU", "base")  # base: c0->ACT, c1 parity, c2->DVE
COPY_SPLIT = os.environ.get("K_COPY", "alt")   # copyout engine: dve|act|alt
LEAD_P = int(os.environ.get("K_LEAD", "5"))    # P-quarters of head start
PF_ENG = os.environ.get("K_PFENG", "gpsimd")   # xt prefetch queue: gpsimd|sync
CREDIT_NUM = 16  # M stages per 6 P units

_NC_CACHE = {}


def _segs():
    """Per projection d-tile: (psum_row, head, head_row, nrows) segments
    mapping packed d rows (128*dt + p) onto per-head (h, q<96) layout."""
    segs = []
    for dt in range(DT):
        cur, d0, d1 = [], 128 * dt, 128 * (dt + 1)
        d = d0
        while d < d1:
            h, q = d // DH, d % DH
            n = min(d1 - d, DH - q)
            cur.append((d - d0, h, q, n))
            d += n
        segs.append(cur)
    return segs


def _dts_of():
    return {h: sorted({(DH * h) // 128, (DH * h + DH - 1) // 128})
            for h in range(NH)}


def _build_nc():
    import concourse.bacc as bacc
    import concourse.tile as tile
    from concourse import mybir

    f32 = mybir.dt.float32
    bf16 = mybir.dt.bfloat16
    AF = mybir.ActivationFunctionType
    OP = mybir.AluOpType
    AX = mybir.AxisListType

    nc = bacc.Bacc()
    xt = nc.declare_dram_parameter("xt", [BPC, KT, 128, S], bf16, isOutput=False)
    p_l = nc.declare_dram_parameter("p_l", [128, KT, NH * DH], bf16, isOutput=False)
    w1 = nc.declare_dram_parameter("w1", [97, NH, DHID], bf16, isOutput=False)
    w2 = nc.declare_dram_parameter("w2", [128, NH, KC, DH], bf16, isOutput=False)
    w2b = nc.declare_dram_parameter("w2b", [DH, NH], f32, isOutput=False)
    pbh = nc.declare_dram_parameter("pbh", [DH, BPC * NH], f32, isOutput=False)
    ones = nc.declare_dram_parameter("ones", [1, NH, S], bf16, isOutput=False)
    out = nc.declare_dram_parameter("out", [DH, BPC * NH], f32, isOutput=True)

    segs = _segs()
    dts_of = _dts_of()

    pf_eng_name = PF_ENG

    with tile.TileContext(nc) as tc:
        with ExitStack() as ctx:
            singles = ctx.enter_context(tc.tile_pool(name="singles", bufs=1))
            xt_pool = ctx.enter_context(tc.tile_pool(name="xtp", bufs=2))
            flat_pool = ctx.enter_context(tc.tile_pool(name="flat", bufs=6))
            a1sb_pool = ctx.enter_context(tc.tile_pool(name="a1sb", bufs=4))
            e_pool = ctx.enter_context(tc.tile_pool(name="ep", bufs=10))
            stt_pool = ctx.enter_context(tc.tile_pool(name="sttp", bufs=3))
            pp_pool = ctx.enter_context(tc.tile_pool(name="pp", bufs=2, space="PSUM"))
            a1p_pool = ctx.enter_context(tc.tile_pool(name="a1p", bufs=4, space="PSUM"))
            a2p_pool = ctx.enter_context(tc.tile_pool(name="a2p", bufs=2, space="PSUM"))

            pf = nc.gpsimd if pf_eng_name == "gpsimd" else nc.sync

            # PE warmup: dense N=512 dummy matmuls (trips the HAM clock gate
            # to 2.4GHz in ~3.4us) while the first batch's DMAs stream in.
            warm_a = singles.tile([128, 128], bf16)
            warm_b = singles.tile([128, SCW], bf16)
            nc.gpsimd.memset(warm_a, 0.0)
            nc.gpsimd.memset(warm_b, 0.0)
            for i in range(N_WARM):
                wp = pp_pool.tile([128, SCW], f32, tag="pp")
                nc.tensor.matmul(wp, warm_a, warm_b, start=True, stop=True)

            # Projection inputs first (needed immediately): p_l per-kt
            # interleaved with batch 0's sc=0 column chunks.
            p_sb = singles.tile([128, KT, NH * DH], bf16)
            xt_t0 = xt_pool.tile([128, KT, S], bf16, tag="xt")
            nc.sync.dma_start(out=p_sb, in_=p_l[:])
            for kt in range(KT):
                nc.sync.dma_start(
                    out=xt_t0[:, kt, 0:2 * SCW], in_=xt[0, kt, :, 0:2 * SCW]
                )

            # MLP-phase constants (needed ~20us in).
            w1_sb = singles.tile([97, NH, DHID], bf16)
            nc.sync.dma_start(out=w1_sb, in_=w1[:])
            w2_sb = singles.tile([128, NH, KC, DH], bf16)
            nc.sync.dma_start(out=w2_sb, in_=w2[:])
            # batch 0's second halves next — needed by sweep 2 (~40us)
            for kt in range(KT):
                nc.sync.dma_start(
                    out=xt_t0[:, kt, 2 * SCW:S], in_=xt[0, kt, :, 2 * SCW:S]
                )
            w2b_sb = singles.tile([DH, NH], f32)
            nc.sync.dma_start(out=w2b_sb, in_=w2b[:])
            pbh_sb = singles.tile([DH, BPC * NH], f32)
            nc.sync.dma_start(out=pbh_sb, in_=pbh[:])


            # Hi^T in per-head layout; row 96 is a constant ones row that
            # realizes the (folded) W1 bias as a 97th contraction row.
            hh = []
            for i in range(2):
                t = singles.tile([97, NH, S], bf16, tag=f"hh{i}")
                nc.sync.dma_start(out=t[96:97, :, :], in_=ones[:])
                hh.append(t)

            # Then all of batch 1 — same sync queue, FIFO keeps priority.
            xt_t1 = xt_pool.tile([128, KT, S], bf16, tag="xt", name="xt_t1")
            for kt in range(KT):
                nc.sync.dma_start(out=xt_t1[:, kt, :], in_=xt[1, kt])
            # (registered below once xt_tiles exists)

            # Per-(b,h) softmax partials, reduced per batch as it finishes.
            z_all = singles.tile([DH, BPC * NH, SC], f32)
            u_all = singles.tile([DH, BPC * NH, 2], f32)
            z_red = singles.tile([DH, BPC * NH], f32)
            u_red = singles.tile([DH, BPC * NH], f32)
            out_sb = singles.tile([DH, BPC * NH], f32)

            xt_tiles = {0: xt_t0, 1: xt_t1}

            def emit_P(b, sc, dt):
                ps = pp_pool.tile([128, SCW], f32, tag="pp")
                xtt = xt_tiles[b]
                for kt in range(KT):
                    nc.tensor.matmul(
                        ps,
                        p_sb[:, kt, 128 * dt:128 * (dt + 1)],
                        xtt[:, kt, SCW * sc:SCW * (sc + 1)],
                        start=(kt == 0),
                        stop=(kt == KT - 1),
                    )
                flat_t = flat_pool.tile([128, SCW], bf16, tag="flat")
                on_act = (dt % 2 == 0) if COPY_SPLIT == "alt" else (COPY_SPLIT == "act")
                if on_act:
                    nc.scalar.copy(out=flat_t, in_=ps)
                else:
                    nc.vector.tensor_copy(flat_t, ps)
                hcur = hh[b % 2]
                for (r0, h, q0, n) in segs[dt]:
                    nc.gpsimd.dma_start(
                        out=hcur[q0:q0 + n, h, SCW * sc:SCW * (sc + 1)],
                        in_=flat_t[r0:r0 + n, :],
                    )

            def emit_M1(b, h, sc):
                hcur = hh[b % 2]
                a1sb = a1sb_pool.tile([128, KC, SCW], bf16, tag="a1sb")
                for c in range(KC):
                    a1p = a1p_pool.tile([128, SCW], f32, tag="a1p")
                    nc.tensor.matmul(
                        a1p,
                        w1_sb[:, h, 128 * c:128 * (c + 1)],
                        hcur[:, h, SCW * sc:SCW * (sc + 1)],
                        start=True,
                        stop=True,
                    )
                    if c == 0:
                        on_act = True
                    elif c == 2:
                        on_act = False
                    else:
                        on_act = h % 2 == 0
                    if on_act:
                        nc.scalar.activation(
                            out=a1sb[:, c, :], in_=a1p, func=AF.Relu
                        )
                    else:
                        nc.vector.tensor_scalar_max(
                            out=a1sb[:, c, :], in0=a1p, scalar1=0.0
                        )
                return a1sb

            e_pairs = {}

            def emit_M2(b, h, sc, a1sb):
                hcur = hh[b % 2]
                col = b * NH + h
                pair = sc // 2
                a2p = a2p_pool.tile([DH, SCW], f32, tag="a2p")
                for kc in range(KC):
                    nc.tensor.matmul(
                        a2p,
                        w2_sb[:, h, kc, :],
                        a1sb[:, kc, :],
                        start=(kc == 0),
                        stop=(kc == KC - 1),
                    )
                if sc % 2 == 0:
                    e_pairs[(b, h, pair)] = e_pool.tile(
                        [DH, 2, SCW], bf16, tag="e_t", name="e_t"
                    )
                e_t = e_pairs[(b, h, pair)]
                nc.scalar.activation(
                    out=e_t[:, sc % 2, :],
                    in_=a2p,
                    func=AF.Exp,
                    bias=w2b_sb[:, h:h + 1],
                    accum_out=z_all[:, col, sc:sc + 1],
                )
                if sc % 2 == 1:
                    stt_t = stt_pool.tile([DH, 2, SCW], bf16, tag="stt_t")
                    nc.vector.scalar_tensor_tensor(
                        out=stt_t,
                        in0=e_t,
                        scalar=1.0,
                        in1=hcur[0:DH, h, SCW * 2 * pair:SCW * 2 * (pair + 1)],
                        op0=OP.mult,
                        op1=OP.mult,
                        accum_out=u_all[:, col, pair:pair + 1],
                    )
                    del e_pairs[(b, h, pair)]

            def emit_batch_finish(b):
                """v = u/Z + P_b for batch b's 8 columns."""
                c0, c1 = b * NH, (b + 1) * NH
                nc.vector.tensor_reduce(
                    out=z_red[:, c0:c1], in_=z_all[:, c0:c1, :],
                    axis=AX.X, op=OP.add,
                )
                nc.vector.tensor_reduce(
                    out=u_red[:, c0:c1], in_=u_all[:, c0:c1, :],
                    axis=AX.X, op=OP.add,
                )
                nc.vector.reciprocal(z_red[:, c0:c1], z_red[:, c0:c1])
                nc.vector.tensor_mul(
                    u_red[:, c0:c1], u_red[:, c0:c1], z_red[:, c0:c1]
                )
                nc.vector.tensor_add(
                    out_sb[:, c0:c1], u_red[:, c0:c1], pbh_sb[:, c0:c1]
                )

            # --- credit-paced scheduler ---------------------------------
            # P units in (b, sc, dt) order; M stage-units (b, h, sc) unlock
            # per-sc as their remaps are emitted. M2(u) trails M1(u) by one
            # stage (software pipelining) so relu->W2 latency is hidden.
            P_units = [(b, sc, dt) for b in range(BPC) for sc in range(SC)
                       for dt in range(DT)]
            remap_done = set()
            m1q = []     # ready M1 stage-units
            m2q = []     # (unit, a1sb) whose M1 is emitted
            m_order = [(b, sc, h) for b in range(BPC) for sc in range(SC)
                       for h in range(NH)]
            m_next = 0

            def refresh_m1q():
                nonlocal m_next
                while m_next < len(m_order):
                    b, sc, h = m_order[m_next]
                    if all((b, dt2, sc) in remap_done for dt2 in dts_of[h]):
                        m1q.append((b, h, sc))
                        m_next += 1
                    else:
                        break

            m2_done = {b: 0 for b in range(BPC)}

            def emit_stage():
                if m2q and (len(m2q) >= 2 or not m1q):
                    (u, a1sb) = m2q.pop(0)
                    emit_M2(*u, a1sb)
                    m2_done[u[0]] += 1
                    if m2_done[u[0]] == NH * SC:
                        emit_batch_finish(u[0])
                    return True
                if m1q:
                    u = m1q.pop(0)
                    a1sb = emit_M1(*u)
                    m2q.append((u, a1sb))
                    return True
                return False

            credit = -CREDIT_NUM * LEAD_P / 6.0
            for pi, (b, sc, dt) in enumerate(P_units):
                u_in_b = sc * DT + dt
                emit_P(b, sc, dt)
                # prefetch batches 2..: 6 full-row (4KB-line) DMAs spread
                # across the previous batch, on the sync queue
                if 1 <= b < BPC - 1 and u_in_b >= 6 and (u_in_b - 6) % 3 == 0:
                    if u_in_b == 6:
                        xt_tiles[b + 1] = xt_pool.tile(
                            [128, KT, S], bf16, tag="xt", name="xt_n"
                        )
                    kt_c = (u_in_b - 6) // 3
                    nc.sync.dma_start(
                        out=xt_tiles[b + 1][:, kt_c, :], in_=xt[b + 1, kt_c]
                    )
                remap_done.add((b, dt, sc))
                refresh_m1q()
                if pi >= len(P_units) - 6:
                    credit = 1e9
                else:
                    credit += CREDIT_NUM / 6.0
                while credit >= 1.0 and emit_stage():
                    credit -= 1.0
            while emit_stage():
                pass
            nc.sync.dma_start(out=out[:], in_=out_sb)
    nc.compile()
    return nc


def get_nc():
    if "nc" not in _NC_CACHE:
        _NC_CACHE["nc"] = _build_nc()
    return _NC_CACHE["nc"]


def make_in_maps(token_embeddings, P_w, P_b, W1_w, W1_b, W2_w, W2_b):
    import ml_dtypes

    bf16 = ml_dtypes.bfloat16
    X = np.asarray(token_embeddings, dtype=np.float32)
    # X^T per batch: (B, T, S) -> tiles [b, kt, p, s] (4KB contiguous rows)
    XT = np.ascontiguousarray(X.transpose(0, 2, 1)).astype(bf16)
    XT = XT.reshape(B, KT, 128, S)

    P_cat = np.transpose(np.asarray(P_w, np.float32), (1, 0, 2)).reshape(T, NH * DH)
    p_l = np.ascontiguousarray(
        P_cat.reshape(KT, 128, NH * DH).transpose(1, 0, 2)
    ).astype(bf16)

    # W1 bias row absorbs P_b: A1 = (X@P)@W1 + (W1_b + P_b@W1)
    Pb = np.asarray(P_b, np.float32)
    W1 = np.asarray(W1_w, np.float32)
    w1 = np.zeros((97, NH, DHID), dtype=bf16)
    w1[:96] = W1.transpose(1, 0, 2).astype(bf16)
    w1[96] = (np.asarray(W1_b, np.float32)
              + np.einsum("hd,hde->he", Pb, W1)).astype(bf16)

    w2 = np.ascontiguousarray(
        np.asarray(W2_w, np.float32).reshape(NH, KC, 128, DH).transpose(2, 0, 1, 3)
    ).astype(bf16)

    w2b = np.ascontiguousarray(np.asarray(W2_b, np.float32).T)
    # v = u/Z + P_b (softmax weights sum to 1): per-core [DH, BPC*NH]
    pbh = np.ascontiguousarray(np.tile(Pb.T, (1, BPC)))
    ones = np.ones((1, NH, S), dtype=bf16)

    in_maps = []
    for c in range(NCORES):
        in_maps.append({
            "xt": np.ascontiguousarray(XT[c * BPC:(c + 1) * BPC]),
            "p_l": p_l,
            "w1": w1,
            "w2": w2,
            "w2b": w2b,
            "pbh": pbh,
            "ones": ones,
        })
    return in_maps


def gather_outputs(res):
    """Per-core out is [DH, BPC*NH]; transpose+reshape to [BPC, NH*DH]."""
    outs = [
        np.ascontiguousarray(np.asarray(r["out"], np.float32).T).reshape(
            BPC, NH * DH
        )
        for r in res.results
    ]
    return np.concatenate(outs, axis=0)


def _reference_host(token_embeddings, attention_mask, P_w, P_b, W1_w, W1_b, W2_w, W2_b):
    """Exact numpy fallback (only used if the mask is not all-ones)."""
    X = np.asarray(token_embeddings, np.float64)
    Hi = np.einsum("bst,htd->bhsd", X, np.asarray(P_w, np.float64))
    Hi += np.asarray(P_b, np.float64)[None, :, None, :]
    A = np.einsum("bhsd,hde->bhse", Hi, np.asarray(W1_w, np.float64))
    A += np.asarray(W1_b, np.float64)[None, :, None, :]
    A = np.maximum(A, 0.0)
    A = np.einsum("bhse,hed->bhsd", A, np.asarray(W2_w, np.float64))
    A += np.asarray(W2_b, np.float64)[None, :, None, :]
    with np.errstate(divide="ignore"):
        logm = np.log(np.asarray(attention_mask, np.float64))[:, None, :, None]
    A = A + logm
    A = A - A.max(axis=2, keepdims=True)
    E = np.exp(A)
    A = E / E.sum(axis=2, keepdims=True)
    v = (Hi * A).sum(axis=2)
    return v.reshape(v.shape[0], NH * DH).astype(np.float32)


def kernel(**inputs):
    mask = np.asarray(inputs["attention_mask"], np.float32)
    if not np.all(mask == 1.0):
        return _reference_host(**inputs)

    from concourse.bass_utils import run_bass_kernel_spmd

    nc = get_nc()
    in_maps = make_in_maps(
        inputs["token_embeddings"], inputs["P_w"], inputs["P_b"],
        inputs["W1_w"], inputs["W1_b"], inputs["W2_w"], inputs["W2_b"],
    )
    res = run_bass_kernel_spmd(nc, in_maps, core_ids=list(range(NCORES)))
    return gather_outputs(res)


# revision 24
# speedup vs baseline: 1.0097x; 1.0097x over previous
"""MultiHeadGeneralizedPooling Trainium2 kernel.

Data-parallel over batch: 32 batches -> 8 cores x 4 batches.
Per core, everything is computed in "feature-major" layout (feature dim on
SBUF partitions, sequence on the free axis):

  Hi^T (d, s)  = P_cat^T @ X^T        TensorE bf16, PSUM (no bias: P_b folded)
  A1^T (dh, s) = relu(W1aug^T @ Hi^T) K=97 (97th row = ones -> W1_b + P_b@W1)
  A2^T (d, s)  = W2^T @ A1^T          accumulated over 3 k-tiles
  E            = exp(A2^T + W2_b)     ScalarE; accum_out -> Z partial
  u[d]         = sum_s E * Hi         sc-pair scalar_tensor_tensor accum
  v            = u / Z + P_b          tiny (96, 32) chain at the end

Schedule: sc-major projection sweeps; remap (flat -> per-head layout) via
SBUF->SBUF DMA at (dt, sc) granularity so MLP quarters unlock per-sc; MLP
units are stage-split (W1+relu | W2+exp+stt) and software-pipelined; xt
prefetch rides the gpsimd DMA queue so remaps never queue behind it.
Output is [96, 32] (host transposes).
"""

import numpy as np
from contextlib import ExitStack

B, S, T = 32, 2048, 768
NH, DH, DHID = 8, 96, 384
NCORES = 8
BPC = B // NCORES  # batches per core
KT = T // 128      # 6 contraction tiles
DT = (NH * DH) // 128  # 6 d-tiles of the packed head dim
SC = 4             # s-chunks per batch
SCW = S // SC      # 512
KC = DHID // 128   # 3
import os
N_WARM = int(os.environ.get("K_NWARM", "42"))  # PE clock-gate warmup matmuls
RELU_SPLIT = os.environ.get("K_RELU", "base")  # base: c0->ACT, c1 parity, c2->DVE
COPY_SPLIT = os.environ.get("K_COPY", "alt")   # copyout engine: dve|act|alt
LEAD_P = int(os.environ.get("K_LEAD", "5"))    # P-quarters of head start
PF_ENG = os.environ.get("K_PFENG", "gpsimd")   # xt prefetch queue: gpsimd|sync
CREDIT_NUM = 16  # M stages per 6 P units

_NC_CACHE = {}


def _segs():
    """Per projection d-tile: (psum_row, head, head_row, nrows) segments
    mapping packed d rows (128*dt + p) onto per-head (h, q<96) layout."""
    segs = []
    for dt in range(DT):
        cur, d0, d1 = [], 128 * dt, 128 * (dt + 1)
        d = d0
        while d < d1:
            h, q = d // DH, d % DH
            n = min(d1 - d, DH - q)
            cur.append((d - d0, h, q, n))
            d += n
        segs.append(cur)
    return segs


def _dts_of():
    return {h: sorted({(DH * h) // 128, (DH * h + DH - 1) // 128})
            for h in range(NH)}


def _build_nc():
    import concourse.bacc as bacc
    import concourse.tile as tile
    from concourse import mybir

    f32 = mybir.dt.float32
    bf16 = mybir.dt.bfloat16
    AF = mybir.ActivationFunctionType
    OP = mybir.AluOpType
    AX = mybir.AxisListType

    nc = bacc.Bacc()
    xt = nc.declare_dram_parameter("xt", [BPC, KT, 128, S], bf16, isOutput=False)
    p_l = nc.declare_dram_parameter("p_l", [128, KT, NH * DH], bf16, isOutput=False)
    w1 = nc.declare_dram_parameter("w1", [97, NH, DHID], bf16, isOutput=False)
    w2 = nc.declare_dram_parameter("w2", [128, NH, KC, DH], bf16, isOutput=False)
    w2b = nc.declare_dram_parameter("w2b", [DH, NH], f32, isOutput=False)
    pbh = nc.declare_dram_parameter("pbh", [DH, BPC * NH], f32, isOutput=False)
    ones = nc.declare_dram_parameter("ones", [1, NH, S], bf16, isOutput=False)
    out = nc.declare_dram_parameter("out", [DH, BPC * NH], f32, isOutput=True)

    segs = _segs()
    dts_of = _dts_of()

    pf_eng_name = PF_ENG

    with tile.TileContext(nc) as tc:
        with ExitStack() as ctx:
            singles = ctx.enter_context(tc.tile_pool(name="singles", bufs=1))
            xt_pool = ctx.enter_context(tc.tile_pool(name="xtp", bufs=2))
            flat_pool = ctx.enter_context(tc.tile_pool(name="flat", bufs=6))
            a1sb_pool = ctx.enter_context(tc.tile_pool(name="a1sb", bufs=4))
            e_pool = ctx.enter_context(tc.tile_pool(name="ep", bufs=10))
            stt_pool = ctx.enter_context(tc.tile_pool(name="sttp", bufs=3))
            pp_pool = ctx.enter_context(tc.tile_pool(name="pp", bufs=2, space="PSUM"))
            a1p_pool = ctx.enter_context(tc.tile_pool(name="a1p", bufs=4, space="PSUM"))
            a2p_pool = ctx.enter_context(tc.tile_pool(name="a2p", bufs=2, space="PSUM"))

            pf = nc.gpsimd if pf_eng_name == "gpsimd" else nc.sync

            # PE warmup: dense N=512 dummy matmuls (trips the HAM clock gate
            # to 2.4GHz in ~3.4us) while the first batch's DMAs stream in.
            warm_a = singles.tile([128, 128], bf16)
            warm_b = singles.tile([128, SCW], bf16)
            nc.gpsimd.memset(warm_a, 0.0)
            nc.gpsimd.memset(warm_b, 0.0)
            for i in range(N_WARM):
                wp = pp_pool.tile([128, SCW], f32, tag="pp")
                nc.tensor.matmul(wp, warm_a, warm_b, start=True, stop=True)

            # Projection inputs first (needed immediately): p_l per-kt
            # interleaved with batch 0's sc=0 column chunks.
            p_sb = singles.tile([128, KT, NH * DH], bf16)
            xt_t0 = xt_pool.tile([128, KT, S], bf16, tag="xt")
            nc.sync.dma_start(out=p_sb, in_=p_l[:])
            for kt in range(KT):
                nc.sync.dma_start(
                    out=xt_t0[:, kt, 0:2 * SCW], in_=xt[0, kt, :, 0:2 * SCW]
                )

            # batch 0's second halves immediately after the first: they must
            # clear the queue before the remap traffic ramps (~24us)
            for kt in range(KT):
                nc.sync.dma_start(
                    out=xt_t0[:, kt, 2 * SCW:S], in_=xt[0, kt, :, 2 * SCW:S]
                )
            # MLP-phase constants (needed ~26us in).
            w1_sb = singles.tile([97, NH, DHID], bf16)
            nc.sync.dma_start(out=w1_sb, in_=w1[:])
            w2_sb = singles.tile([128, NH, KC, DH], bf16)
            nc.sync.dma_start(out=w2_sb, in_=w2[:])
            w2b_sb = singles.tile([DH, NH], f32)
            nc.sync.dma_start(out=w2b_sb, in_=w2b[:])
            pbh_sb = singles.tile([DH, BPC * NH], f32)
            nc.sync.dma_start(out=pbh_sb, in_=pbh[:])


            # Hi^T in per-head layout; row 96 is a constant ones row that
            # realizes the (folded) W1 bias as a 97th contraction row.
            hh = []
            for i in range(2):
                t = singles.tile([97, NH, S], bf16, tag=f"hh{i}")
                nc.sync.dma_start(out=t[96:97, :, :], in_=ones[:])
                hh.append(t)

            # Then all of batch 1 — same sync queue, FIFO keeps priority.
            xt_t1 = xt_pool.tile([128, KT, S], bf16, tag="xt", name="xt_t1")
            for kt in range(KT):
                nc.sync.dma_start(out=xt_t1[:, kt, :], in_=xt[1, kt])
            # (registered below once xt_tiles exists)

            # Per-(b,h) softmax partials, reduced per batch as it finishes.
            z_all = singles.tile([DH, BPC * NH, SC], f32)
            u_all = singles.tile([DH, BPC * NH, 2], f32)
            z_red = singles.tile([DH, BPC * NH], f32)
            u_red = singles.tile([DH, BPC * NH], f32)
            out_sb = singles.tile([DH, BPC * NH], f32)

            xt_tiles = {0: xt_t0, 1: xt_t1}

            def emit_P(b, sc, dt):
                ps = pp_pool.tile([128, SCW], f32, tag="pp")
                xtt = xt_tiles[b]
                for kt in range(KT):
                    nc.tensor.matmul(
                        ps,
                        p_sb[:, kt, 128 * dt:128 * (dt + 1)],
                        xtt[:, kt, SCW * sc:SCW * (sc + 1)],
                        start=(kt == 0),
                        stop=(kt == KT - 1),
                    )
                flat_t = flat_pool.tile([128, SCW], bf16, tag="flat")
                on_act = (dt % 2 == 0) if COPY_SPLIT == "alt" else (COPY_SPLIT == "act")
                if on_act:
                    nc.scalar.copy(out=flat_t, in_=ps)
                else:
                    nc.vector.tensor_copy(flat_t, ps)
                hcur = hh[b % 2]
                for (r0, h, q0, n) in segs[dt]:
                    nc.gpsimd.dma_start(
                        out=hcur[q0:q0 + n, h, SCW * sc:SCW * (sc + 1)],
                        in_=flat_t[r0:r0 + n, :],
                    )

            def emit_M1(b, h, sc):
                hcur = hh[b % 2]
                a1sb = a1sb_pool.tile([128, KC, SCW], bf16, tag="a1sb")
                for c in range(KC):
                    a1p = a1p_pool.tile([128, SCW], f32, tag="a1p")
                    nc.tensor.matmul(
                        a1p,
                        w1_sb[:, h, 128 * c:128 * (c + 1)],
                        hcur[:, h, SCW * sc:SCW * (sc + 1)],
                        start=True,
                        stop=True,
                    )
                    if c == 0:
                        on_act = True
                    elif c == 2:
                        on_act = False
                    else:
                        on_act = h % 2 == 0
                    if on_act:
                        nc.scalar.activation(
                            out=a1sb[:, c, :], in_=a1p, func=AF.Relu
                        )
                    else:
                        nc.vector.tensor_scalar_max(
                            out=a1sb[:, c, :], in0=a1p, scalar1=0.0
                        )
                return a1sb

            e_pairs = {}

            def emit_M2(b, h, sc, a1sb):
                hcur = hh[b % 2]
                col = b * NH + h
                pair = sc // 2
                a2p = a2p_pool.tile([DH, SCW], f32, tag="a2p")
                for kc in range(KC):
                    nc.tensor.matmul(
                        a2p,
                        w2_sb[:, h, kc, :],
                        a1sb[:, kc, :],
                        start=(kc == 0),
                        stop=(kc == KC - 1),
                    )
                if sc % 2 == 0:
                    e_pairs[(b, h, pair)] = e_pool.tile(
                        [DH, 2, SCW], bf16, tag="e_t", name="e_t"
                    )
                e_t = e_pairs[(b, h, pair)]
                nc.scalar.activation(
                    out=e_t[:, sc % 2, :],
                    in_=a2p,
                    func=AF.Exp,
                    bias=w2b_sb[:, h:h + 1],
                    accum_out=z_all[:, col, sc:sc + 1],
                )
                if sc % 2 == 1:
                    stt_t = stt_pool.tile([DH, 2, SCW], bf16, tag="stt_t")
                    nc.vector.scalar_tensor_tensor(
                        out=stt_t,
                        in0=e_t,
                        scalar=1.0,
                        in1=hcur[0:DH, h, SCW * 2 * pair:SCW * 2 * (pair + 1)],
                        op0=OP.mult,
                        op1=OP.mult,
                        accum_out=u_all[:, col, pair:pair + 1],
                    )
                    del e_pairs[(b, h, pair)]

            def emit_batch_finish(b):
                """v = u/Z + P_b for batch b's 8 columns."""
                c0, c1 = b * NH, (b + 1) * NH
                nc.vector.tensor_reduce(
                    out=z_red[:, c0:c1], in_=z_all[:, c0:c1, :],
                    axis=AX.X, op=OP.add,
                )
                nc.vector.tensor_reduce(
                    out=u_red[:, c0:c1], in_=u_all[:, c0:c1, :],
                    axis=AX.X, op=OP.add,
                )
                nc.vector.reciprocal(z_red[:, c0:c1], z_red[:, c0:c1])
                nc.vector.tensor_mul(
                    u_red[:, c0:c1], u_red[:, c0:c1], z_red[:, c0:c1]
                )
                nc.vector.tensor_add(
                    out_sb[:, c0:c1], u_red[:, c0:c1], pbh_sb[:, c0:c1]
                )

            # --- credit-paced scheduler ---------------------------------
            # P units in (b, sc, dt) order; M stage-units (b, h, sc) unlock
            # per-sc as their remaps are emitted. M2(u) trails M1(u) by one
            # stage (software pipelining) so relu->W2 latency is hidden.
            P_units = [(b, sc, dt) for b in range(BPC) for sc in range(SC)
                       for dt in range(DT)]
            remap_done = set()
            m1q = []     # ready M1 stage-units
            m2q = []     # (unit, a1sb) whose M1 is emitted
            m_order = [(b, sc, h) for b in range(BPC) for sc in range(SC)
                       for h in range(NH)]
            m_next = 0

            def refresh_m1q():
                nonlocal m_next
                while m_next < len(m_order):
                    b, sc, h = m_order[m_next]
                    if all((b, dt2, sc) in remap_done for dt2 in dts_of[h]):
                        m1q.append((b, h, sc))
                        m_next += 1
                    else:
                        break

            m2_done = {b: 0 for b in range(BPC)}

            def emit_stage():
                if m2q and (len(m2q) >= 2 or not m1q):
                    (u, a1sb) = m2q.pop(0)
                    emit_M2(*u, a1sb)
                    m2_done[u[0]] += 1
                    if m2_done[u[0]] == NH * SC:
                        emit_batch_finish(u[0])
                    return True
                if m1q:
                    u = m1q.pop(0)
                    a1sb = emit_M1(*u)
                    m2q.append((u, a1sb))
                    return True
                return False

            credit = -CREDIT_NUM * LEAD_P / 6.0
            for pi, (b, sc, dt) in enumerate(P_units):
                u_in_b = sc * DT + dt
                emit_P(b, sc, dt)
                # prefetch batches 2..: 6 full-row (4KB-line) DMAs spread
                # across the previous batch, on the sync queue
                if 1 <= b < BPC - 1 and u_in_b >= 6 and (u_in_b - 6) % 3 == 0:
                    if u_in_b == 6:
                        xt_tiles[b + 1] = xt_pool.tile(
                            [128, KT, S], bf16, tag="xt", name="xt_n"
                        )
                    kt_c = (u_in_b - 6) // 3
                    nc.sync.dma_start(
                        out=xt_tiles[b + 1][:, kt_c, :], in_=xt[b + 1, kt_c]
                    )
                remap_done.add((b, dt, sc))
                refresh_m1q()
                if pi >= len(P_units) - 6:
                    credit = 1e9
                else:
                    credit += CREDIT_NUM / 6.0
                while credit >= 1.0 and emit_stage():
                    credit -= 1.0
            while emit_stage():
                pass
            nc.sync.dma_start(out=out[:], in_=out_sb)
    nc.compile()
    return nc


def get_nc():
    if "nc" not in _NC_CACHE:
        _NC_CACHE["nc"] = _build_nc()
    return _NC_CACHE["nc"]


def make_in_maps(token_embeddings, P_w, P_b, W1_w, W1_b, W2_w, W2_b):
    import ml_dtypes

    bf16 = ml_dtypes.bfloat16
    X = np.asarray(token_embeddings, dtype=np.float32)
    # X^T per batch: (B, T, S) -> tiles [b, kt, p, s] (4KB contiguous rows)
    XT = np.ascontiguousarray(X.transpose(0, 2, 1)).astype(bf16)
    XT = XT.reshape(B, KT, 128, S)

    P_cat = np.transpose(np.asarray(P_w, np.float32), (1, 0, 2)).reshape(T, NH * DH)
    p_l = np.ascontiguousarray(
        P_cat.reshape(KT, 128, NH * DH).transpose(1, 0, 2)
    ).astype(bf16)

    # W1 bias row absorbs P_b: A1 = (X@P)@W1 + (W1_b + P_b@W1)
    Pb = np.asarray(P_b, np.float32)
    W1 = np.asarray(W1_w, np.float32)
    w1 = np.zeros((97, NH, DHID), dtype=bf16)
    w1[:96] = W1.transpose(1, 0, 2).astype(bf16)
    w1[96] = (np.asarray(W1_b, np.float32)
              + np.einsum("hd,hde->he", Pb, W1)).astype(bf16)

    w2 = np.ascontiguousarray(
        np.asarray(W2_w, np.float32).reshape(NH, KC, 128, DH).transpose(2, 0, 1, 3)
    ).astype(bf16)

    w2b = np.ascontiguousarray(np.asarray(W2_b, np.float32).T)
    # v = u/Z + P_b (softmax weights sum to 1): per-core [DH, BPC*NH]
    pbh = np.ascontiguousarray(np.tile(Pb.T, (1, BPC)))
    ones = np.ones((1, NH, S), dtype=bf16)

    in_maps = []
    for c in range(NCORES):
        in_maps.append({
            "xt": np.ascontiguousarray(XT[c * BPC:(c + 1) * BPC]),
            "p_l": p_l,
            "w1": w1,
            "w2": w2,
            "w2b": w2b,
            "pbh": pbh,
            "ones": ones,
        })
    return in_maps


def gather_outputs(res):
    """Per-core out is [DH, BPC*NH]; transpose+reshape to [BPC, NH*DH]."""
    outs = [
        np.ascontiguousarray(np.asarray(r["out"], np.float32).T).reshape(
            BPC, NH * DH
        )
        for r in res.results
    ]
    return np.concatenate(outs, axis=0)


def _reference_host(token_embeddings, attention_mask, P_w, P_b, W1_w, W1_b, W2_w, W2_b):
    """Exact numpy fallback (only used if the mask is not all-ones)."""
    X = np.asarray(token_embeddings, np.float64)
    Hi = np.einsum("bst,htd->bhsd", X, np.asarray(P_w, np.float64))
    Hi += np.asarray(P_b, np.float64)[None, :, None, :]
    A = np.einsum("bhsd,hde->bhse", Hi, np.asarray(W1_w, np.float64))
    A += np.asarray(W1_b, np.float64)[None, :, None, :]
    A = np.maximum(A, 0.0)
    A = np.einsum("bhse,hed->bhsd", A, np.asarray(W2_w, np.float64))
    A += np.asarray(W2_b, np.float64)[None, :, None, :]
    with np.errstate(divide="ignore"):
        logm = np.log(np.asarray(attention_mask, np.float64))[:, None, :, None]
    A = A + logm
    A = A - A.max(axis=2, keepdims=True)
    E = np.exp(A)
    A = E / E.sum(axis=2, keepdims=True)
    v = (Hi * A).sum(axis=2)
    return v.reshape(v.shape[0], NH * DH).astype(np.float32)


def kernel(**inputs):
    mask = np.asarray(inputs["attention_mask"], np.float32)
    if not np.all(mask == 1.0):
        return _reference_host(**inputs)

    from concourse.bass_utils import run_bass_kernel_spmd

    nc = get_nc()
    in_maps = make_in_maps(
        inputs["token_embeddings"], inputs["P_w"], inputs["P_b"],
        inputs["W1_w"], inputs["W1_b"], inputs["W2_w"], inputs["W2_b"],
    )
    res = run_bass_kernel_spmd(nc, in_maps, core_ids=list(range(NCORES)))
    return gather_outputs(res)


# revision 25
# speedup vs baseline: 1.0137x; 1.0039x over previous
"""MultiHeadGeneralizedPooling Trainium2 kernel.

Data-parallel over batch: 32 batches -> 8 cores x 4 batches.
Per core, everything is computed in "feature-major" layout (feature dim on
SBUF partitions, sequence on the free axis):

  Hi^T (d, s)  = P_cat^T @ X^T        TensorE bf16, PSUM (no bias: P_b folded)
  A1^T (dh, s) = relu(W1aug^T @ Hi^T) K=97 (97th row = ones -> W1_b + P_b@W1)
  A2^T (d, s)  = W2^T @ A1^T          accumulated over 3 k-tiles
  E            = exp(A2^T + W2_b)     ScalarE; accum_out -> Z partial
  u[d]         = sum_s E * Hi         sc-pair scalar_tensor_tensor accum
  v            = u / Z + P_b          tiny (96, 32) chain at the end

Schedule: sc-major projection sweeps; remap (flat -> per-head layout) via
SBUF->SBUF DMA at (dt, sc) granularity so MLP quarters unlock per-sc; MLP
units are stage-split (W1+relu | W2+exp+stt) and software-pipelined; xt
prefetch rides the gpsimd DMA queue so remaps never queue behind it.
Output is [96, 32] (host transposes).
"""

import numpy as np
from contextlib import ExitStack

B, S, T = 32, 2048, 768
NH, DH, DHID = 8, 96, 384
NCORES = 8
BPC = B // NCORES  # batches per core
KT = T // 128      # 6 contraction tiles
DT = (NH * DH) // 128  # 6 d-tiles of the packed head dim
SC = 4             # s-chunks per batch
SCW = S // SC      # 512
KC = DHID // 128   # 3
import os
N_WARM = int(os.environ.get("K_NWARM", "42"))  # PE clock-gate warmup matmuls
RELU_SPLIT = os.environ.get("K_RELU", "base")  # base: c0->ACT, c1 parity, c2->DVE
COPY_SPLIT = os.environ.get("K_COPY", "alt")   # copyout engine: dve|act|alt
LEAD_P = int(os.environ.get("K_LEAD", "5"))    # P-quarters of head start
PF_ENG = os.environ.get("K_PFENG", "gpsimd")   # xt prefetch queue: gpsimd|sync
CREDIT_NUM = 16  # M stages per 6 P units

_NC_CACHE = {}


def _segs():
    """Per projection d-tile: (psum_row, head, head_row, nrows) segments
    mapping packed d rows (128*dt + p) onto per-head (h, q<96) layout."""
    segs = []
    for dt in range(DT):
        cur, d0, d1 = [], 128 * dt, 128 * (dt + 1)
        d = d0
        while d < d1:
            h, q = d // DH, d % DH
            n = min(d1 - d, DH - q)
            cur.append((d - d0, h, q, n))
            d += n
        segs.append(cur)
    return segs


def _dts_of():
    return {h: sorted({(DH * h) // 128, (DH * h + DH - 1) // 128})
            for h in range(NH)}


def _build_nc():
    import concourse.bacc as bacc
    import concourse.tile as tile
    from concourse import mybir

    f32 = mybir.dt.float32
    bf16 = mybir.dt.bfloat16
    AF = mybir.ActivationFunctionType
    OP = mybir.AluOpType
    AX = mybir.AxisListType

    nc = bacc.Bacc()
    xt = nc.declare_dram_parameter("xt", [BPC, KT, 128, S], bf16, isOutput=False)
    p_l = nc.declare_dram_parameter("p_l", [128, KT, NH * DH], bf16, isOutput=False)
    w1 = nc.declare_dram_parameter("w1", [97, NH, DHID], bf16, isOutput=False)
    w2 = nc.declare_dram_parameter("w2", [128, NH, KC, DH], bf16, isOutput=False)
    w2b = nc.declare_dram_parameter("w2b", [DH, NH], f32, isOutput=False)
    pbh = nc.declare_dram_parameter("pbh", [DH, BPC * NH], f32, isOutput=False)
    ones = nc.declare_dram_parameter("ones", [1, NH, S], bf16, isOutput=False)
    out = nc.declare_dram_parameter("out", [DH, BPC * NH], f32, isOutput=True)

    segs = _segs()
    dts_of = _dts_of()

    pf_eng_name = PF_ENG

    with tile.TileContext(nc) as tc:
        with ExitStack() as ctx:
            singles = ctx.enter_context(tc.tile_pool(name="singles", bufs=1))
            xt_pool = ctx.enter_context(tc.tile_pool(name="xtp", bufs=2))
            flat_pool = ctx.enter_context(tc.tile_pool(name="flat", bufs=6))
            a1sb_pool = ctx.enter_context(tc.tile_pool(name="a1sb", bufs=4))
            e_pool = ctx.enter_context(tc.tile_pool(name="ep", bufs=10))
            stt_pool = ctx.enter_context(tc.tile_pool(name="sttp", bufs=3))
            pp_pool = ctx.enter_context(tc.tile_pool(name="pp", bufs=2, space="PSUM"))
            a1p_pool = ctx.enter_context(tc.tile_pool(name="a1p", bufs=4, space="PSUM"))
            a2p_pool = ctx.enter_context(tc.tile_pool(name="a2p", bufs=2, space="PSUM"))

            pf = nc.gpsimd if pf_eng_name == "gpsimd" else nc.sync

            # PE warmup: dense N=512 dummy matmuls (trips the HAM clock gate
            # to 2.4GHz in ~3.4us) while the first batch's DMAs stream in.
            warm_a = singles.tile([128, 128], bf16)
            warm_b = singles.tile([128, SCW], bf16)
            nc.gpsimd.memset(warm_a, 0.0)
            nc.gpsimd.memset(warm_b, 0.0)
            for i in range(N_WARM):
                wp = pp_pool.tile([128, SCW], f32, tag="pp")
                nc.tensor.matmul(wp, warm_a, warm_b, start=True, stop=True)

            # Projection inputs first (needed immediately): p_l per-kt
            # interleaved with batch 0's sc=0 column chunks.
            p_sb = singles.tile([128, KT, NH * DH], bf16)
            xt_t0 = xt_pool.tile([128, KT, S], bf16, tag="xt")
            nc.sync.dma_start(out=p_sb, in_=p_l[:])
            for kt in range(KT):
                nc.sync.dma_start(
                    out=xt_t0[:, kt, 0:2 * SCW], in_=xt[0, kt, :, 0:2 * SCW]
                )

            # batch 0's second halves immediately after the first: they must
            # clear the queue before the remap traffic ramps (~24us)
            for kt in range(KT):
                nc.sync.dma_start(
                    out=xt_t0[:, kt, 2 * SCW:S], in_=xt[0, kt, :, 2 * SCW:S]
                )
            # MLP-phase constants (needed ~26us in).
            w1_sb = singles.tile([97, NH, DHID], bf16)
            nc.sync.dma_start(out=w1_sb, in_=w1[:])
            w2_sb = singles.tile([128, NH, KC, DH], bf16)
            nc.sync.dma_start(out=w2_sb, in_=w2[:])
            w2b_sb = singles.tile([DH, NH], f32)
            nc.sync.dma_start(out=w2b_sb, in_=w2b[:])
            pbh_sb = singles.tile([DH, BPC * NH], f32)
            nc.sync.dma_start(out=pbh_sb, in_=pbh[:])


            # Hi^T in per-head layout; row 96 is a constant ones row that
            # realizes the (folded) W1 bias as a 97th contraction row.
            hh = []
            for i in range(2):
                t = singles.tile([97, NH, S], bf16, tag=f"hh{i}")
                nc.sync.dma_start(out=t[96:97, :, :], in_=ones[:])
                hh.append(t)



            # Per-(b,h) softmax partials, reduced per batch as it finishes.
            z_all = singles.tile([DH, BPC * NH, SC], f32)
            u_all = singles.tile([DH, BPC * NH, 2], f32)
            z_red = singles.tile([DH, BPC * NH], f32)
            u_red = singles.tile([DH, BPC * NH], f32)
            out_sb = singles.tile([DH, BPC * NH], f32)

            xt_tiles = {0: xt_t0}

            def emit_P(b, sc, dt):
                ps = pp_pool.tile([128, SCW], f32, tag="pp")
                xtt = xt_tiles[b]
                for kt in range(KT):
                    nc.tensor.matmul(
                        ps,
                        p_sb[:, kt, 128 * dt:128 * (dt + 1)],
                        xtt[:, kt, SCW * sc:SCW * (sc + 1)],
                        start=(kt == 0),
                        stop=(kt == KT - 1),
                    )
                flat_t = flat_pool.tile([128, SCW], bf16, tag="flat")
                on_act = (dt % 2 == 0) if COPY_SPLIT == "alt" else (COPY_SPLIT == "act")
                if on_act:
                    nc.scalar.copy(out=flat_t, in_=ps)
                else:
                    nc.vector.tensor_copy(flat_t, ps)
                hcur = hh[b % 2]
                for (r0, h, q0, n) in segs[dt]:
                    nc.sync.dma_start(
                        out=hcur[q0:q0 + n, h, SCW * sc:SCW * (sc + 1)],
                        in_=flat_t[r0:r0 + n, :],
                    )

            def emit_M1(b, h, sc):
                hcur = hh[b % 2]
                a1sb = a1sb_pool.tile([128, KC, SCW], bf16, tag="a1sb")
                for c in range(KC):
                    a1p = a1p_pool.tile([128, SCW], f32, tag="a1p")
                    nc.tensor.matmul(
                        a1p,
                        w1_sb[:, h, 128 * c:128 * (c + 1)],
                        hcur[:, h, SCW * sc:SCW * (sc + 1)],
                        start=True,
                        stop=True,
                    )
                    if c == 0:
                        on_act = True
                    elif c == 2:
                        on_act = False
                    else:
                        on_act = h % 2 == 0
                    if on_act:
                        nc.scalar.activation(
                            out=a1sb[:, c, :], in_=a1p, func=AF.Relu
                        )
                    else:
                        nc.vector.tensor_scalar_max(
                            out=a1sb[:, c, :], in0=a1p, scalar1=0.0
                        )
                return a1sb

            e_pairs = {}

            def emit_M2(b, h, sc, a1sb):
                hcur = hh[b % 2]
                col = b * NH + h
                pair = sc // 2
                a2p = a2p_pool.tile([DH, SCW], f32, tag="a2p")
                for kc in range(KC):
                    nc.tensor.matmul(
                        a2p,
                        w2_sb[:, h, kc, :],
                        a1sb[:, kc, :],
                        start=(kc == 0),
                        stop=(kc == KC - 1),
                    )
                if sc % 2 == 0:
                    e_pairs[(b, h, pair)] = e_pool.tile(
                        [DH, 2, SCW], bf16, tag="e_t", name="e_t"
                    )
                e_t = e_pairs[(b, h, pair)]
                nc.scalar.activation(
                    out=e_t[:, sc % 2, :],
                    in_=a2p,
                    func=AF.Exp,
                    bias=w2b_sb[:, h:h + 1],
                    accum_out=z_all[:, col, sc:sc + 1],
                )
                if sc % 2 == 1:
                    stt_t = stt_pool.tile([DH, 2, SCW], bf16, tag="stt_t")
                    nc.vector.scalar_tensor_tensor(
                        out=stt_t,
                        in0=e_t,
                        scalar=1.0,
                        in1=hcur[0:DH, h, SCW * 2 * pair:SCW * 2 * (pair + 1)],
                        op0=OP.mult,
                        op1=OP.mult,
                        accum_out=u_all[:, col, pair:pair + 1],
                    )
                    del e_pairs[(b, h, pair)]

            def emit_batch_finish(b):
                """v = u/Z + P_b for batch b's 8 columns."""
                c0, c1 = b * NH, (b + 1) * NH
                nc.vector.tensor_reduce(
                    out=z_red[:, c0:c1], in_=z_all[:, c0:c1, :],
                    axis=AX.X, op=OP.add,
                )
                nc.vector.tensor_reduce(
                    out=u_red[:, c0:c1], in_=u_all[:, c0:c1, :],
                    axis=AX.X, op=OP.add,
                )
                nc.vector.reciprocal(z_red[:, c0:c1], z_red[:, c0:c1])
                nc.vector.tensor_mul(
                    u_red[:, c0:c1], u_red[:, c0:c1], z_red[:, c0:c1]
                )
                nc.vector.tensor_add(
                    out_sb[:, c0:c1], u_red[:, c0:c1], pbh_sb[:, c0:c1]
                )

            # --- credit-paced scheduler ---------------------------------
            # P units in (b, sc, dt) order; M stage-units (b, h, sc) unlock
            # per-sc as their remaps are emitted. M2(u) trails M1(u) by one
            # stage (software pipelining) so relu->W2 latency is hidden.
            P_units = [(b, sc, dt) for b in range(BPC) for sc in range(SC)
                       for dt in range(DT)]
            remap_done = set()
            m1q = []     # ready M1 stage-units
            m2q = []     # (unit, a1sb) whose M1 is emitted
            m_order = [(b, sc, h) for b in range(BPC) for sc in range(SC)
                       for h in range(NH)]
            m_next = 0

            def refresh_m1q():
                nonlocal m_next
                while m_next < len(m_order):
                    b, sc, h = m_order[m_next]
                    if all((b, dt2, sc) in remap_done for dt2 in dts_of[h]):
                        m1q.append((b, h, sc))
                        m_next += 1
                    else:
                        break

            m2_done = {b: 0 for b in range(BPC)}

            def emit_stage():
                if m2q and (len(m2q) >= 2 or not m1q):
                    (u, a1sb) = m2q.pop(0)
                    emit_M2(*u, a1sb)
                    m2_done[u[0]] += 1
                    if m2_done[u[0]] == NH * SC:
                        emit_batch_finish(u[0])
                    return True
                if m1q:
                    u = m1q.pop(0)
                    a1sb = emit_M1(*u)
                    m2q.append((u, a1sb))
                    return True
                return False

            credit = -CREDIT_NUM * LEAD_P / 6.0
            for pi, (b, sc, dt) in enumerate(P_units):
                u_in_b = sc * DT + dt
                emit_P(b, sc, dt)
                # prefetch the next batch: 6 full-row (4KB-line) DMAs
                # spread across this batch, on the same sync queue
                if b < BPC - 1 and u_in_b >= 6 and (u_in_b - 6) % 3 == 0:
                    if u_in_b == 6:
                        xt_tiles[b + 1] = xt_pool.tile(
                            [128, KT, S], bf16, tag="xt", name="xt_n"
                        )
                    kt_c = (u_in_b - 6) // 3
                    nc.sync.dma_start(
                        out=xt_tiles[b + 1][:, kt_c, :], in_=xt[b + 1, kt_c]
                    )
                remap_done.add((b, dt, sc))
                refresh_m1q()
                if pi >= len(P_units) - 6:
                    credit = 1e9
                else:
                    credit += CREDIT_NUM / 6.0
                while credit >= 1.0 and emit_stage():
                    credit -= 1.0
            while emit_stage():
                pass
            nc.sync.dma_start(out=out[:], in_=out_sb)
    nc.compile()
    return nc


def get_nc():
    if "nc" not in _NC_CACHE:
        _NC_CACHE["nc"] = _build_nc()
    return _NC_CACHE["nc"]


def make_in_maps(token_embeddings, P_w, P_b, W1_w, W1_b, W2_w, W2_b):
    import ml_dtypes

    bf16 = ml_dtypes.bfloat16
    X = np.asarray(token_embeddings, dtype=np.float32)
    # X^T per batch: (B, T, S) -> tiles [b, kt, p, s] (4KB contiguous rows)
    XT = np.ascontiguousarray(X.transpose(0, 2, 1)).astype(bf16)
    XT = XT.reshape(B, KT, 128, S)

    P_cat = np.transpose(np.asarray(P_w, np.float32), (1, 0, 2)).reshape(T, NH * DH)
    p_l = np.ascontiguousarray(
        P_cat.reshape(KT, 128, NH * DH).transpose(1, 0, 2)
    ).astype(bf16)

    # W1 bias row absorbs P_b: A1 = (X@P)@W1 + (W1_b + P_b@W1)
    Pb = np.asarray(P_b, np.float32)
    W1 = np.asarray(W1_w, np.float32)
    w1 = np.zeros((97, NH, DHID), dtype=bf16)
    w1[:96] = W1.transpose(1, 0, 2).astype(bf16)
    w1[96] = (np.asarray(W1_b, np.float32)
              + np.einsum("hd,hde->he", Pb, W1)).astype(bf16)

    w2 = np.ascontiguousarray(
        np.asarray(W2_w, np.float32).reshape(NH, KC, 128, DH).transpose(2, 0, 1, 3)
    ).astype(bf16)

    w2b = np.ascontiguousarray(np.asarray(W2_b, np.float32).T)
    # v = u/Z + P_b (softmax weights sum to 1): per-core [DH, BPC*NH]
    pbh = np.ascontiguousarray(np.tile(Pb.T, (1, BPC)))
    ones = np.ones((1, NH, S), dtype=bf16)

    in_maps = []
    for c in range(NCORES):
        in_maps.append({
            "xt": np.ascontiguousarray(XT[c * BPC:(c + 1) * BPC]),
            "p_l": p_l,
            "w1": w1,
            "w2": w2,
            "w2b": w2b,
            "pbh": pbh,
            "ones": ones,
        })
    return in_maps


def gather_outputs(res):
    """Per-core out is [DH, BPC*NH]; transpose+reshape to [BPC, NH*DH]."""
    outs = [
        np.ascontiguousarray(np.asarray(r["out"], np.float32).T).reshape(
            BPC, NH * DH
        )
        for r in res.results
    ]
    return np.concatenate(outs, axis=0)


def _reference_host(token_embeddings, attention_mask, P_w, P_b, W1_w, W1_b, W2_w, W2_b):
    """Exact numpy fallback (only used if the mask is not all-ones)."""
    X = np.asarray(token_embeddings, np.float64)
    Hi = np.einsum("bst,htd->bhsd", X, np.asarray(P_w, np.float64))
    Hi += np.asarray(P_b, np.float64)[None, :, None, :]
    A = np.einsum("bhsd,hde->bhse", Hi, np.asarray(W1_w, np.float64))
    A += np.asarray(W1_b, np.float64)[None, :, None, :]
    A = np.maximum(A, 0.0)
    A = np.einsum("bhse,hed->bhsd", A, np.asarray(W2_w, np.float64))
    A += np.asarray(W2_b, np.float64)[None, :, None, :]
    with np.errstate(divide="ignore"):
        logm = np.log(np.asarray(attention_mask, np.float64))[:, None, :, None]
    A = A + logm
    A = A - A.max(axis=2, keepdims=True)
    E = np.exp(A)
    A = E / E.sum(axis=2, keepdims=True)
    v = (Hi * A).sum(axis=2)
    return v.reshape(v.shape[0], NH * DH).astype(np.float32)


def kernel(**inputs):
    mask = np.asarray(inputs["attention_mask"], np.float32)
    if not np.all(mask == 1.0):
        return _reference_host(**inputs)

    from concourse.bass_utils import run_bass_kernel_spmd

    nc = get_nc()
    in_maps = make_in_maps(
        inputs["token_embeddings"], inputs["P_w"], inputs["P_b"],
        inputs["W1_w"], inputs["W1_b"], inputs["W2_w"], inputs["W2_b"],
    )
    res = run_bass_kernel_spmd(nc, in_maps, core_ids=list(range(NCORES)))
    return gather_outputs(res)
